# revision 45
# baseline (speedup 1.0000x reference)
"""Trainium2 Bass kernel for AttentionBlock (B=4, C=256, H=W=64).

Sharding: 8 cores = (batch b, query-half h). Each core holds the full
x[b] (for K over all 4096 key positions) and computes the attention
output for its 2048 query positions. The host permutes x columns so the
core's own query half comes first (key/value order is irrelevant:
softmax and the value contraction sum over all j). The host also
supplies xT (x transposed, bf16) so the value contraction needs no
on-chip transposes.

Per-core dataflow (Tile framework, one NeuronCore):
  warmup: dummy matmuls during the initial DMA window ramp the PE
  p-state; a dummy activation preloads the ACT exp table.
  qk = WqkT.T @ x[:, blk] + bqk       packed q|k projection [64, 512]
  query superblocks of width 512,512,512,256,256 (the two final minis
  halve the end-of-kernel tail chain), software-pipelined with each
  other and with the projections. Per superblock, for each j-group
  (4 chunks of 128 keys):
      eT[j, i] = k_chunk.T @ q_blk     (PE -> PSUM f32, 2+2 chunks in
                                        ping-pong half-tiles so exp
                                        never blocks the next energy)
      ex = exp(eT)                     (ACT, PSUM->SBUF, bf16)
      pair/quad partial sums on DVE (bf16 2x mode); quads of group
      pairs (0,1)(2,3)(4,5) are oct-combined, groups 6,7 stay quads;
      the 5 ones-matmuls are deferred via a pending queue so they
      never stall the in-order PE ahead of z work
      z[cin, i] += xT_chunk.T @ ex     (PE bf16; reassociated value
                                        path: out = Wv (x attn))
  tail: recip of sums on DVE (gamma is folded into Wv host-side),
  Pool broadcast, fused zs = z * (1/sums) on evacuation; for the last
  superblock the residual x + gamma*bv rides the PE via an identity
  matmul and the output evacuates on the ACT engine so the final
  serial chain stays off the DVE.
Notes:
 - softmax rows sum to 1, so the v-bias contributes exactly gamma*bv[c]
   to the output; z is computed bias-free and bv folds into the final
   elementwise op.
 - softmax runs without max subtraction: energies are in [-45, 42] for
   this input distribution, well inside f32 exp range; exp is stored as
   bf16 (range is fine, ~0.4% rounding) which keeps the z matmuls at
   full PE rate and halves the DVE pair-add cost.
 - f32 matmul operands use float32r (full-rate fp32 matmul on TRN2).
"""

import numpy as np
import ml_dtypes

import concourse.bass as bass
import concourse.mybir as mybir
import concourse.tile as tile
from concourse import bacc
from concourse.bass_utils import run_bass_kernel_spmd

AF = mybir.ActivationFunctionType
OP = mybir.AluOpType
F32 = mybir.dt.float32
F32R = mybir.dt.float32r
BF16 = mybir.dt.bfloat16

B, C, HH, WW = 4, 256, 64, 64
N = HH * WW          # 4096 spatial positions
CQ = 32              # q/k channels
NCORES = 8
NQ = N // 2          # 2048 queries per core
P = 128
FB = 512             # free-dim block (one PSUM bank of f32)
HF = 256             # mini-superblock width for the final two blocks
JCH = N // P         # 32 j-chunks
NCH = C // P         # 2 channel chunks
GRP = 4              # j-chunks per energy/exp group
NG = JCH // GRP      # 8 groups per superblock
NWARM = 5            # PE warmup matmuls during the head DMA window
CPACK = 260          # const-pack: wqk(128) bqk(1) pad(1) bvg(2) ident(128)
SBS = [(0, FB), (FB, FB), (2 * FB, FB), (3 * FB, HF), (3 * FB + HF, HF)]


def _emit_body(nc, tc, d):
    """Emit one full forward pass. d: dict of DRAM APs."""
    with (
        tc.tile_pool(name="const", bufs=1) as cpool,
        tc.tile_pool(name="xp", bufs=1) as xpool,
        tc.tile_pool(name="kq", bufs=1) as kqpool,
    ):
        # ---- x: [128, 2, 4096] (channel chunks interleaved per
        #      partition); first 512-col block split per chunk so the
        #      first projection starts ASAP, then the packed constants ----
        x_sb = xpool.tile([P, NCH, N], F32R, tag="x", name="x")
        for cc in range(NCH):
            nc.sync.dma_start(x_sb[:, cc, 0:FB], d["x"][:, cc, 0:FB])

        cst = cpool.tile([P, CPACK], F32R, tag="cst", name="cst")
        nc.sync.dma_start(cst[:], d["cst"][:])
        wqk_sb = [cst[:, 0:2 * CQ], cst[:, 2 * CQ:4 * CQ]]
        bqk_sb = cst[0:2 * CQ, 128:129].bitcast(F32)
        bv_sb = [cst[:, 130:131].bitcast(F32), cst[:, 131:132].bitcast(F32)]
        ident_sb = cst[:, 132:260]
        ones_sb = cpool.tile([P, 1], BF16, tag="ones")
        nc.gpsimd.memset(ones_sb[:], 1.0)

        def dma_x(nb, split=False):
            sl = bass.ts(nb, FB)
            if split:
                for cc in range(NCH):
                    nc.sync.dma_start(x_sb[:, cc, sl], d["x"][:, cc, sl])
            else:
                nc.sync.dma_start(x_sb[:, :, sl], d["x"][:, :, sl])

        xt_sb = xpool.tile([P, JCH * C], BF16, tag="xt", name="xt")
        xt_view = d["xT"].rearrange("(a p) c -> p a c", p=P)   # [128, 32, 256]

        def dma_xtq(ab, parts=4):
            w = JCH // parts
            asl = bass.ts(ab, w)
            nc.sync.dma_start(
                xt_sb[:, ab * w * C:(ab + 1) * w * C],
                xt_view[:, asl, :])

        dma_x(1, split=True)
        dma_x(2, split=True)
        dma_x(3, split=True)
        dma_xtq(0, 8)
        dma_x(4)
        dma_xtq(1, 8)
        dma_x(5)
        dma_xtq(2, 8)
        dma_x(6)
        dma_xtq(3, 8)
        dma_x(7)
        dma_xtq(2, 4)
        dma_xtq(3, 4)

        wv_sb = xpool.tile([P, NCH, C], F32R, tag="wv", name="wv")
        nc.sync.dma_start(wv_sb[:], d["wvT"][:])

        # ---- q/k projections + attention ----
        # PSUM: ps_e(4 banks) coexists first with ps_proj(4), then with
        # ps_acc(4) after projections close.
        with (
            tc.tile_pool(name="ex", bufs=4) as expool,
            tc.tile_pool(name="ps_e", bufs=1, space="PSUM") as pse,
        ):
            states = []
            q_sb = kqpool.tile([CQ, NQ], F32R, tag="q")
            k_sb = kqpool.tile([CQ, N], F32R, tag="k")

            with tc.tile_pool(name="fin", bufs=4) as fpool:
                def emit_eexp(state, g):
                    fb = state["fb"]
                    # energy in two ping-pong half-tiles: the exp of half A
                    # overlaps the energy matmuls of half B, and the next
                    # group's energy needn't wait a whole-group exp.
                    ex_halves = []
                    for hh in range(2):
                        pe_t = pse.tile([P, 2 * fb], F32, tag=f"pe{hh}",
                                        name="pe")
                        for jj in range(2):
                            j = GRP * g + 2 * hh + jj
                            nc.tensor.matmul(
                                pe_t[:, bass.ts(jj, fb)],
                                k_sb[:, bass.ts(j, P)],
                                q_sb[:, state["isl"]],
                                start=True, stop=True,
                            )
                        ex_t = expool.tile([P, 2 * fb], BF16, tag=f"ex{hh}",
                                           name="ex")
                        nc.scalar.activation(ex_t[:], pe_t[:], AF.Exp)
                        ex_halves.append(ex_t)
                    state["exps"][g] = ex_halves
                    # bf16 partial sums on DVE (2x mode): pair, quad, then
                    # octs for group pairs (0,1)(2,3)(4,5); groups 6,7 stay
                    # quads. Ones-matmuls are deferred via state["pend"].
                    pr0 = fpool.tile([P, fb], BF16, tag="pr0", name="pr0")
                    nc.vector.tensor_tensor(pr0[:],
                                            ex_halves[0][:, bass.ts(0, fb)],
                                            ex_halves[0][:, bass.ts(1, fb)],
                                            op=OP.add)
                    pr1 = fpool.tile([P, fb], BF16, tag="pr1", name="pr1")
                    nc.vector.tensor_tensor(pr1[:],
                                            ex_halves[1][:, bass.ts(0, fb)],
                                            ex_halves[1][:, bass.ts(1, fb)],
                                            op=OP.add)
                    qd = fpool.tile([P, fb], BF16, tag=f"qd{g % 2}",
                                    name="qd")
                    nc.vector.tensor_tensor(qd[:], pr0[:], pr1[:], op=OP.add)
                    if g >= 6:
                        state["pend"].append((g, qd))
                    elif g % 2 == 1:
                        oc = fpool.tile([P, fb], BF16, tag="oc", name="oc")
                        nc.vector.tensor_tensor(oc[:],
                                                state["quads"].pop(g - 1),
                                                qd[:], op=OP.add)
                        state["pend"].append((g, oc))
                    else:
                        state["quads"][g] = qd

                def flush_pend(state, before_g=None):
                    """Emit deferred ones-matmuls whose reduction tile was
                    created before group `before_g` (None = flush all)."""
                    if not state["pend"]:
                        return
                    if state["sm"] is None:
                        state["sm"] = psacc.tile([P, state["fb"]], F32,
                                                 tag="smops", name="smops")
                    keep = []
                    for cg, t in state["pend"]:
                        if before_g is not None and cg >= before_g:
                            keep.append((cg, t))
                            continue
                        nc.tensor.matmul(
                            state["sm"][0:1, :], ones_sb[:, 0:1], t[:],
                            start=(state["nones"] == 0),
                            stop=(state["nones"] == 4),
                        )
                        state["nones"] += 1
                    state["pend"] = keep

                def new_state(isb):
                    i0, fb = SBS[isb]
                    return {"isl": slice(i0, i0 + fb), "i0": i0, "fb": fb,
                            "z": None, "sm": None, "exps": {}, "quads": {},
                            "pend": [], "nones": 0, "zs": None, "bc": None,
                            "rg": None, "xrb": None,
                            "last": isb == len(SBS) - 1}

                def proj_qk(nb, pool, tag):
                    """Packed q|k projection for x block nb (q rows 0:32,
                    k rows 32:64 of the PSUM output)."""
                    ps = pool.tile([P, FB], F32, tag=tag,
                                   name="psp")[0:2 * CQ, :]
                    for cc in range(NCH):
                        nc.tensor.matmul(
                            ps[:], wqk_sb[cc], x_sb[:, cc, bass.ts(nb, FB)],
                            start=(cc == 0), stop=(cc == NCH - 1),
                        )
                    nc.vector.tensor_scalar(q_sb[:, bass.ts(nb, FB)],
                                            ps[0:CQ, :], bqk_sb[0:CQ, 0:1],
                                            None, op0=OP.add)
                    nc.vector.tensor_scalar(k_sb[:, bass.ts(nb, FB)],
                                            ps[CQ:2 * CQ, :],
                                            bqk_sb[CQ:2 * CQ, 0:1],
                                            None, op0=OP.add)

                def proj_k(nb, pool, tag):
                    """k-only projection for x block nb (blocks 4-7)."""
                    ps = pool.tile([P, FB], F32, tag=tag, name="psp")[0:CQ, :]
                    for cc in range(NCH):
                        nc.tensor.matmul(
                            ps[:], wqk_sb[cc][:, CQ:2 * CQ],
                            x_sb[:, cc, bass.ts(nb, FB)],
                            start=(cc == 0), stop=(cc == NCH - 1),
                        )
                    nc.vector.tensor_scalar(k_sb[:, bass.ts(nb, FB)], ps[:],
                                            bqk_sb[CQ:2 * CQ, 0:1],
                                            None, op0=OP.add)

                state0 = new_state(0)
                states.append(state0)
                with tc.tile_pool(name="ps_proj", bufs=4,
                                  space="PSUM") as psproj:
                    # PE p-state warmup + ACT exp-table preload: dummy ops
                    # on a zeroed tile while the first x slices are in
                    # flight. The first 4 energy groups interleave with the
                    # projections so the PE never queues behind a
                    # DMA-blocked projection.
                    wu_sb = fpool.tile([P, FB], BF16, tag="wu", name="wu")
                    nc.gpsimd.memset(wu_sb[:], 0.0)
                    wact = fpool.tile([1, 1], F32, tag="wact", name="wact")
                    nc.scalar.activation(wact[:], wu_sb[0:1, 0:1], AF.Exp)
                    wps = psproj.tile([P, FB], F32, tag="psp", name="wps")
                    for _ in range(NWARM):
                        nc.tensor.matmul(wps[:], wu_sb[:, 0:P], wu_sb[:],
                                         start=True, stop=True)
                    for nb in range(4):
                        proj_qk(nb, psproj, "psp")
                        emit_eexp(state0, nb)

                def emit_zg(state, g, jjs=None):
                    ccmajor = jjs is None and g == NG - 1
                    if jjs is None:
                        ex_h = state["exps"].pop(g)
                        jjs = range(GRP)
                    else:
                        ex_h = state["exps"][g]
                        if jjs[-1] == GRP - 1:
                            state["exps"].pop(g)
                    fb = state["fb"]
                    if ccmajor:
                        # cc-major: finish the z0 accumulator a few matmuls
                        # early so its evacuation/out-projection chain
                        # starts sooner at the superblock tail
                        for cc in range(NCH):
                            for jj in jjs:
                                j = GRP * g + jj
                                nc.tensor.matmul(
                                    state["z"][cc][:],
                                    xt_sb[:, j * C + cc * P:
                                          j * C + (cc + 1) * P],
                                    ex_h[jj // 2][:, bass.ts(jj % 2, fb)],
                                    start=(j == 0), stop=(j == JCH - 1),
                                )
                        return
                    for jj in jjs:
                        j = GRP * g + jj
                        exsl = ex_h[jj // 2][:, bass.ts(jj % 2, fb)]
                        for cc in range(NCH):
                            nc.tensor.matmul(
                                state["z"][cc][:],
                                xt_sb[:, j * C + cc * P: j * C + (cc + 1) * P],
                                exsl,
                                start=(j == 0), stop=(j == JCH - 1),
                            )

                def tail_recip(state):
                    """1/sums on DVE (gamma is folded into wvT host-side)."""
                    recip_sb = fpool.tile([1, state["fb"]], F32, tag="recip",
                                          name="recip")
                    nc.vector.reciprocal(recip_sb[:], state["sm"][0:1, :])
                    state["rg"] = recip_sb

                def emit_xrb(state):
                    """Residual x + gamma*bv, computed off the critical path;
                    added into the out-projection PSUM via an identity
                    matmul so the final evacuation is a plain ACT copy."""
                    xrb = fpool.tile([P, NCH, state["fb"]], F32R, tag="xrb",
                                     name="xrb")
                    for cc in range(NCH):
                        nc.vector.tensor_scalar(
                            xrb[:, cc, :],
                            x_sb[:, cc, state["isl"]].bitcast(F32),
                            bv_sb[cc][:, 0:1], None, op0=OP.add)
                    state["xrb"] = xrb

                def tail_bc(state):
                    """Broadcast 1/sums to 128 partitions (Pool; SBUF output
                    so the fused zs-scale keeps a single PSUM operand)."""
                    bc_sb = fpool.tile([P, state["fb"]], F32, tag="bc_sb",
                                       name="bc_sb")
                    nc.gpsimd.partition_broadcast(bc_sb[:],
                                                  state["rg"][0:1, :])
                    state["bc"] = bc_sb

                def tail_zs(state):
                    """Fused evacuate+normalize: zs = z * (1/sums)."""
                    state["zs"] = []
                    for cc in range(NCH):
                        t = fpool.tile([P, state["fb"]], F32R, tag=f"zs{cc}",
                                       name=f"zs{cc}")
                        nc.vector.tensor_tensor(t[:], state["z"][cc][:],
                                                state["bc"][:], op=OP.mult)
                        state["zs"].append(t)

                def tail_b(state, last=False):
                    fb = state["fb"]
                    i0 = state["i0"]
                    for co in range(NCH):
                        if co == 1:
                            if last:
                                ops = pse.tile([P, 2 * fb], F32, tag="pe0",
                                               name="opsl")[:, 0:fb]
                            else:
                                ops = psacc.tile([P, fb], F32, tag="smops",
                                                 name="ops2")
                        else:
                            ops = psacc.tile([P, fb], F32, tag="ops",
                                             name="ops")
                        if last:
                            # residual rides the PE; evacuation on the ACT
                            # engine (same table as Exp) keeps the final
                            # serial chain off the DVE
                            nc.tensor.matmul(ops[:], ident_sb,
                                             state["xrb"][:, co, :],
                                             start=True, stop=False)
                        for ci in range(NCH):
                            nc.tensor.matmul(
                                ops[:],
                                wv_sb[:, ci, co * P:(co + 1) * P],
                                state["zs"][ci][:],
                                start=(ci == 0 and not last),
                                stop=(ci == NCH - 1),
                            )
                        o_sb = fpool.tile([P, fb], F32, tag=f"osb{co}",
                                          name="osb")
                        if last:
                            nc.scalar.activation(o_sb[:], ops[:], AF.Copy)
                        else:
                            nc.vector.scalar_tensor_tensor(
                                o_sb[:], ops[:], bv_sb[co][:, 0:1],
                                x_sb[:, co, state["isl"]].bitcast(F32),
                                op0=OP.add, op1=OP.add,
                            )
                        nc.sync.dma_start(
                            d["out"][co * P:(co + 1) * P, i0:i0 + fb],
                            o_sb[:])

                with tc.tile_pool(name="ps_acc", bufs=1,
                                  space="PSUM") as psacc:
                    for isb in range(len(SBS)):
                        if isb == 0:
                            state = states[0]
                        else:
                            state = new_state(isb)
                            states.append(state)
                        state["z"] = [
                            psacc.tile([P, state["fb"]], F32, tag=f"z{cc}",
                                       name=f"z{cc}")
                            for cc in range(NCH)]
                        for g in range(NG):
                            if isb == 0:
                                # groups 0-3 were emitted with the
                                # projections; bodies 0-3 consume their z
                                # and run the remaining k-projections
                                if g < 4:
                                    emit_zg(state, g)
                                    proj_k(4 + g, psacc, "ops")
                                else:
                                    emit_eexp(state, g)
                                    flush_pend(state, g)
                                    if g >= 5:
                                        emit_zg(state, g - 1)
                                continue
                            emit_eexp(state, g)
                            if state["last"] and g == NG - 1:
                                # final superblock: get the last quad's
                                # ones-matmul onto the PE mid-zg so the
                                # recip/bc chain overlaps the trailing z
                                flush_pend(state, g)
                                emit_zg(state, g - 1, jjs=[0, 1])
                                flush_pend(state)
                                emit_zg(state, g - 1, jjs=[2, 3])
                                continue
                            flush_pend(state, g)
                            prev = states[isb - 1]
                            if g == 0:
                                flush_pend(prev)
                                tail_recip(prev)
                                tail_bc(prev)
                                emit_zg(prev, NG - 1)
                                tail_zs(prev)
                            if g >= 1:
                                emit_zg(state, g - 1)
                            if g == 1:
                                tail_b(prev)
                            if g == 2 and state["last"]:
                                emit_xrb(state)
                    last = states[-1]
                    flush_pend(last)
                    tail_recip(last)
                    tail_bc(last)
                    emit_zg(last, NG - 1)
                    tail_zs(last)
                    tail_b(last, last=True)


_programs = {}


def build_program(repeat=1):
    if repeat in _programs:
        return _programs[repeat]
    nc = bacc.Bacc("TRN2", target_bir_lowering=False, debug=False,
                   num_devices=NCORES)
    d = {
        "x": nc.dram_tensor("x", [P, NCH, N], F32R,
                            kind="ExternalInput").ap(),
        "xT": nc.dram_tensor("xT", [N, C], BF16, kind="ExternalInput").ap(),
        "cst": nc.dram_tensor("cst", [P, CPACK], F32R,
                              kind="ExternalInput").ap(),
        "wvT": nc.dram_tensor("wvT", [P, NCH, C], F32R,
                              kind="ExternalInput").ap(),
        "out": nc.dram_tensor("out", [C, NQ], F32, kind="ExternalOutput").ap(),
    }
    with tile.TileContext(nc) as tc:
        for _ in range(repeat):
            _emit_body(nc, tc, d)
    nc.compile()
    _programs[repeat] = nc
    return nc


def make_in_maps(x, Wq, bq, Wk, bk, Wv, bv, gamma):
    x = np.asarray(x, dtype=np.float32)
    Wq = np.asarray(Wq, dtype=np.float32)
    bq = np.asarray(bq, dtype=np.float32)
    Wk = np.asarray(Wk, dtype=np.float32)
    bk = np.asarray(bk, dtype=np.float32)
    Wv = np.asarray(Wv, dtype=np.float32)
    bv = np.asarray(bv, dtype=np.float32)
    gamma = np.asarray(gamma, dtype=np.float32)

    # const pack: wqk cc0 | wqk cc1 | bqk | pad | bvg0 | bvg1 | identity
    cst = np.zeros((P, CPACK), np.float32)
    wqk = np.concatenate([Wq.T, Wk.T], axis=1)          # [256, 64]
    cst[:, 0:64] = wqk[0:P]
    cst[:, 64:128] = wqk[P:C]
    cst[0:2 * CQ, 128] = np.concatenate([bq, bk])
    bvg = gamma.reshape(()) * bv
    cst[:, 130] = bvg[0:P]
    cst[:, 131] = bvg[P:C]
    cst[:, 132:260] = np.eye(P, dtype=np.float32)

    # gamma folded into the value projection weights
    wvt = np.ascontiguousarray(
        (gamma.reshape(()) * Wv).T
        .reshape(NCH, P, C).transpose(1, 0, 2))          # [128, 2, 256]

    shared = {"cst": cst, "wvT": wvt}
    in_maps = []
    for core in range(NCORES):
        b, h = core // 2, core % 2
        xb = x[b].reshape(C, N)
        xr = np.concatenate(
            [xb[:, h * NQ:(h + 1) * NQ], xb[:, (1 - h) * NQ:(2 - h) * NQ]],
            axis=1)
        m = dict(shared)
        m["x"] = np.ascontiguousarray(
            xr.reshape(NCH, P, N).transpose(1, 0, 2))    # [128, 2, 4096]
        m["xT"] = np.ascontiguousarray(xr.T).astype(ml_dtypes.bfloat16)
        in_maps.append(m)
    return in_maps


def assemble_output(results, dtype=np.float32):
    out = np.empty((B, C, N), np.float32)
    for core in range(NCORES):
        b, h = core // 2, core % 2
        out[b][:, h * NQ:(h + 1) * NQ] = results[core]["out"]
    return out.reshape(B, C, HH, WW).astype(dtype, copy=False)


def kernel(x, Wq, bq, Wk, bk, Wv, bv, gamma):
    nc = build_program(repeat=1)
    in_maps = make_in_maps(x, Wq, bq, Wk, bk, Wv, bv, gamma)
    res = run_bass_kernel_spmd(nc, in_maps, list(range(NCORES)))
    return assemble_output(res.results, dtype=np.asarray(x).dtype)


# revision 46
# speedup vs baseline: 1.0440x; 1.0440x over previous
"""Trainium2 Bass kernel for AttentionBlock (B=4, C=256, H=W=64).

Sharding: 8 cores = (batch b, query-half h). Each core holds the full
x[b] (for K over all 4096 key positions) and computes the attention
output for its 2048 query positions. The host permutes x columns so the
core's own query half comes first (key/value order is irrelevant:
softmax and the value contraction sum over all j). The host also
supplies xT (x transposed, bf16) so the value contraction needs no
on-chip transposes.

Per-core dataflow (Tile framework, one NeuronCore):
  warmup: dummy matmuls during the initial DMA window ramp the PE
  p-state; a dummy activation preloads the ACT exp table.
  qk = WqkT.T @ x[:, blk] + bqk       packed q|k projection [64, 512]
  query superblocks of width 512,512,512,256,256 (the two final minis
  halve the end-of-kernel tail chain), software-pipelined with each
  other and with the projections. Per superblock, for each j-group
  (4 chunks of 128 keys):
      eT[j, i] = k_chunk.T @ q_blk     (PE -> PSUM f32, 2+2 chunks in
                                        ping-pong half-tiles so exp
                                        never blocks the next energy)
      ex = exp(eT)                     (ACT, PSUM->SBUF, bf16)
      pair/quad partial sums on DVE (bf16 2x mode); quads of group
      pairs (0,1)(2,3)(4,5) are oct-combined, groups 6,7 stay quads;
      the 5 ones-matmuls are deferred via a pending queue so they
      never stall the in-order PE ahead of z work
      z[cin, i] += xT_chunk.T @ ex     (PE bf16; reassociated value
                                        path: out = Wv (x attn))
  tail: recip of sums on DVE (gamma is folded into Wv host-side),
  Pool broadcast, fused zs = z * (1/sums) on evacuation; for the last
  superblock the residual x + gamma*bv rides the PE via an identity
  matmul and the output evacuates on the ACT engine so the final
  serial chain stays off the DVE.
Notes:
 - softmax rows sum to 1, so the v-bias contributes exactly gamma*bv[c]
   to the output; z is computed bias-free and bv folds into the final
   elementwise op.
 - softmax runs without max subtraction: energies are in [-45, 42] for
   this input distribution, well inside f32 exp range; exp is stored as
   bf16 (range is fine, ~0.4% rounding) which keeps the z matmuls at
   full PE rate and halves the DVE pair-add cost.
 - f32 matmul operands use float32r (full-rate fp32 matmul on TRN2).
"""

import numpy as np
import ml_dtypes

import concourse.bass as bass
import concourse.mybir as mybir
import concourse.tile as tile
from concourse import bacc
from concourse.bass_utils import run_bass_kernel_spmd

AF = mybir.ActivationFunctionType
OP = mybir.AluOpType
F32 = mybir.dt.float32
F32R = mybir.dt.float32r
BF16 = mybir.dt.bfloat16

B, C, HH, WW = 4, 256, 64, 64
N = HH * WW          # 4096 spatial positions
CQ = 32              # q/k channels
NCORES = 8
NQ = N // 2          # 2048 queries per core
P = 128
FB = 512             # free-dim block (one PSUM bank of f32)
HF = 256             # mini-superblock width for the final two blocks
JCH = N // P         # 32 j-chunks
NCH = C // P         # 2 channel chunks
GRP = 4              # j-chunks per energy/exp group
NG = JCH // GRP      # 8 groups per superblock
NWARM = 7            # PE warmup matmuls during the head DMA window
CPACK = 260          # const-pack: wqk(128) bqk(1) pad(1) bvg(2) ident(128)
SBS = [(0, FB), (FB, FB), (2 * FB, FB), (3 * FB, FB)]


def _emit_body(nc, tc, d):
    """Emit one full forward pass. d: dict of DRAM APs."""
    with (
        tc.tile_pool(name="const", bufs=1) as cpool,
        tc.tile_pool(name="xp", bufs=1) as xpool,
        tc.tile_pool(name="kq", bufs=1) as kqpool,
    ):
        # ---- x: [128, 2, 4096] (channel chunks interleaved per
        #      partition); first 512-col block split per chunk so the
        #      first projection starts ASAP, then the packed constants ----
        x_sb = xpool.tile([P, NCH, N], F32R, tag="x", name="x")
        for cc in range(NCH):
            nc.sync.dma_start(x_sb[:, cc, 0:FB], d["x"][:, cc, 0:FB])

        cst = cpool.tile([P, CPACK], F32R, tag="cst", name="cst")
        nc.sync.dma_start(cst[:], d["cst"][:])
        wqk_sb = [cst[:, 0:2 * CQ], cst[:, 2 * CQ:4 * CQ]]
        bqk_sb = cst[0:2 * CQ, 128:129].bitcast(F32)
        bv_sb = [cst[:, 130:131].bitcast(F32), cst[:, 131:132].bitcast(F32)]
        ident_sb = cst[:, 132:260]
        ones_sb = cpool.tile([P, 1], BF16, tag="ones")
        nc.gpsimd.memset(ones_sb[:], 1.0)

        def dma_x(nb, split=False):
            sl = bass.ts(nb, FB)
            if split:
                for cc in range(NCH):
                    nc.sync.dma_start(x_sb[:, cc, sl], d["x"][:, cc, sl])
            else:
                nc.sync.dma_start(x_sb[:, :, sl], d["x"][:, :, sl])

        xt_sb = xpool.tile([P, JCH * C], BF16, tag="xt", name="xt")
        xt_view = d["xT"].rearrange("(a p) c -> p a c", p=P)   # [128, 32, 256]

        def dma_xtq(ab, parts=4):
            w = JCH // parts
            asl = bass.ts(ab, w)
            nc.sync.dma_start(
                xt_sb[:, ab * w * C:(ab + 1) * w * C],
                xt_view[:, asl, :])

        dma_x(1, split=True)
        dma_x(2, split=True)
        dma_x(3, split=True)
        dma_xtq(0, 8)
        dma_x(4)
        dma_xtq(1, 8)
        dma_x(5)
        dma_xtq(2, 8)
        dma_x(6)
        dma_xtq(3, 8)
        dma_x(7)
        dma_xtq(2, 4)
        dma_xtq(3, 4)

        wv_sb = xpool.tile([P, NCH, C], F32R, tag="wv", name="wv")
        nc.sync.dma_start(wv_sb[:], d["wvT"][:])

        # ---- q/k projections + attention ----
        # PSUM: ps_e(4 banks) coexists first with ps_proj(4), then with
        # ps_acc(4) after projections close.
        with (
            tc.tile_pool(name="ex", bufs=4) as expool,
            tc.tile_pool(name="ps_e", bufs=1, space="PSUM") as pse,
        ):
            states = []
            q_sb = kqpool.tile([CQ, NQ], F32R, tag="q")
            k_sb = kqpool.tile([CQ, N], F32R, tag="k")

            with tc.tile_pool(name="fin", bufs=4) as fpool:
                def emit_eexp(state, g):
                    fb = state["fb"]
                    # energy in two ping-pong half-tiles: the exp of half A
                    # overlaps the energy matmuls of half B, and the next
                    # group's energy needn't wait a whole-group exp.
                    ex_halves = []
                    for hh in range(2):
                        pe_t = pse.tile([P, 2 * fb], F32, tag=f"pe{hh}",
                                        name="pe")
                        for jj in range(2):
                            j = GRP * g + 2 * hh + jj
                            nc.tensor.matmul(
                                pe_t[:, bass.ts(jj, fb)],
                                k_sb[:, bass.ts(j, P)],
                                q_sb[:, state["isl"]],
                                start=True, stop=True,
                            )
                        ex_t = expool.tile([P, 2 * fb], BF16, tag=f"ex{hh}",
                                           name="ex")
                        nc.scalar.activation(ex_t[:], pe_t[:], AF.Exp)
                        ex_halves.append(ex_t)
                    state["exps"][g] = ex_halves
                    # bf16 partial sums on DVE (2x mode): pair, quad, then
                    # octs for group pairs (0,1)(2,3)(4,5); groups 6,7 stay
                    # quads. Ones-matmuls are deferred via state["pend"].
                    pr0 = fpool.tile([P, fb], BF16, tag="pr0", name="pr0")
                    nc.vector.tensor_tensor(pr0[:],
                                            ex_halves[0][:, bass.ts(0, fb)],
                                            ex_halves[0][:, bass.ts(1, fb)],
                                            op=OP.add)
                    pr1 = fpool.tile([P, fb], BF16, tag="pr1", name="pr1")
                    nc.vector.tensor_tensor(pr1[:],
                                            ex_halves[1][:, bass.ts(0, fb)],
                                            ex_halves[1][:, bass.ts(1, fb)],
                                            op=OP.add)
                    qd = fpool.tile([P, fb], BF16, tag=f"qd{g % 2}",
                                    name="qd")
                    nc.vector.tensor_tensor(qd[:], pr0[:], pr1[:], op=OP.add)
                    # unbalanced merge tree: groups 0-6 reduce into ONE root
                    # (ready early, since group 7 isn't in it); quad7 stays
                    # separate so the final sums never wait extra merges.
                    # -> 2 ones-matmuls per superblock.
                    def _merge(a, b, tg):
                        t = fpool.tile([P, fb], BF16, tag=tg, name="mg")
                        nc.vector.tensor_tensor(t[:], a[:], b[:], op=OP.add)
                        return t
                    qs = state["quads"]
                    if g == 7:
                        state["pend"].append((g, qd))
                    elif g in (1, 3, 5):
                        qs[f"m{g - 1}{g}"] = _merge(qs.pop(g - 1), qd,
                                                    f"oc{(g // 2) % 2}")
                        if g == 3:
                            qs["m0123"] = _merge(qs.pop("m01"),
                                                 qs.pop("m23"), "hx")
                    elif g == 6:
                        m456 = _merge(qs.pop("m45"), qd, "oc1")
                        root = _merge(qs.pop("m0123"), m456, "rt")
                        state["pend"].append((g, root))
                    else:
                        qs[g] = qd

                def flush_pend(state, before_g=None):
                    """Emit deferred ones-matmuls whose reduction tile was
                    created before group `before_g` (None = flush all)."""
                    if not state["pend"]:
                        return
                    if state["sm"] is None:
                        state["sm"] = psacc.tile([P, state["fb"]], F32,
                                                 tag="smops", name="smops")
                    keep = []
                    for cg, t in state["pend"]:
                        if before_g is not None and cg >= before_g:
                            keep.append((cg, t))
                            continue
                        nc.tensor.matmul(
                            state["sm"][0:1, :], ones_sb[:, 0:1], t[:],
                            start=(state["nones"] == 0),
                            stop=(state["nones"] == 1),
                        )
                        state["nones"] += 1
                    state["pend"] = keep

                def new_state(isb):
                    i0, fb = SBS[isb]
                    return {"isl": slice(i0, i0 + fb), "i0": i0, "fb": fb,
                            "z": None, "sm": None, "exps": {}, "quads": {},
                            "pend": [], "nones": 0, "zs": None, "bc": None,
                            "rg": None, "xrb": None,
                            "last": isb == len(SBS) - 1}

                def proj_qk(nb, pool, tag):
                    """Packed q|k projection for x block nb (q rows 0:32,
                    k rows 32:64 of the PSUM output)."""
                    ps = pool.tile([P, FB], F32, tag=tag,
                                   name="psp")[0:2 * CQ, :]
                    for cc in range(NCH):
                        nc.tensor.matmul(
                            ps[:], wqk_sb[cc], x_sb[:, cc, bass.ts(nb, FB)],
                            start=(cc == 0), stop=(cc == NCH - 1),
                        )
                    nc.vector.tensor_scalar(q_sb[:, bass.ts(nb, FB)],
                                            ps[0:CQ, :], bqk_sb[0:CQ, 0:1],
                                            None, op0=OP.add)
                    nc.vector.tensor_scalar(k_sb[:, bass.ts(nb, FB)],
                                            ps[CQ:2 * CQ, :],
                                            bqk_sb[CQ:2 * CQ, 0:1],
                                            None, op0=OP.add)

                def proj_k(nb, pool, tag):
                    """k-only projection for x block nb (blocks 4-7)."""
                    ps = pool.tile([P, FB], F32, tag=tag, name="psp")[0:CQ, :]
                    for cc in range(NCH):
                        nc.tensor.matmul(
                            ps[:], wqk_sb[cc][:, CQ:2 * CQ],
                            x_sb[:, cc, bass.ts(nb, FB)],
                            start=(cc == 0), stop=(cc == NCH - 1),
                        )
                    nc.vector.tensor_scalar(k_sb[:, bass.ts(nb, FB)], ps[:],
                                            bqk_sb[CQ:2 * CQ, 0:1],
                                            None, op0=OP.add)

                state0 = new_state(0)
                states.append(state0)
                with tc.tile_pool(name="ps_proj", bufs=4,
                                  space="PSUM") as psproj:
                    # PE p-state warmup + ACT exp-table preload: dummy ops
                    # on a zeroed tile while the first x slices are in
                    # flight. The first 4 energy groups interleave with the
                    # projections so the PE never queues behind a
                    # DMA-blocked projection.
                    wu_sb = fpool.tile([P, FB], BF16, tag="wu", name="wu")
                    nc.gpsimd.memset(wu_sb[:], 0.0)
                    wact = fpool.tile([1, 1], F32, tag="wact", name="wact")
                    nc.scalar.activation(wact[:], wu_sb[0:1, 0:1], AF.Exp)
                    for _ in range(NWARM):
                        wps = psproj.tile([P, FB], F32, tag="psp",
                                          name="wps")
                        nc.tensor.matmul(wps[:], wu_sb[:, 0:P], wu_sb[:],
                                         start=True, stop=True)
                    for nb in range(4):
                        proj_qk(nb, psproj, "psp")
                        emit_eexp(state0, nb)

                def emit_zg(state, g, jjs=None):
                    ccmajor = jjs is None and g == NG - 1
                    if jjs is None:
                        ex_h = state["exps"].pop(g)
                        jjs = range(GRP)
                    else:
                        ex_h = state["exps"][g]
                        if jjs[-1] == GRP - 1:
                            state["exps"].pop(g)
                    fb = state["fb"]
                    if ccmajor:
                        # cc-major: finish the z0 accumulator a few matmuls
                        # early so its evacuation/out-projection chain
                        # starts sooner at the superblock tail
                        for cc in range(NCH):
                            for jj in jjs:
                                j = GRP * g + jj
                                nc.tensor.matmul(
                                    state["z"][cc][:],
                                    xt_sb[:, j * C + cc * P:
                                          j * C + (cc + 1) * P],
                                    ex_h[jj // 2][:, bass.ts(jj % 2, fb)],
                                    start=(j == 0), stop=(j == JCH - 1),
                                )
                        return
                    for jj in jjs:
                        j = GRP * g + jj
                        exsl = ex_h[jj // 2][:, bass.ts(jj % 2, fb)]
                        for cc in range(NCH):
                            nc.tensor.matmul(
                                state["z"][cc][:],
                                xt_sb[:, j * C + cc * P: j * C + (cc + 1) * P],
                                exsl,
                                start=(j == 0), stop=(j == JCH - 1),
                            )

                def tail_recip(state):
                    """1/sums on DVE (gamma is folded into wvT host-side)."""
                    recip_sb = fpool.tile([1, state["fb"]], F32, tag="recip",
                                          name="recip")
                    nc.vector.reciprocal(recip_sb[:], state["sm"][0:1, :])
                    state["rg"] = recip_sb

                def emit_xrb(state):
                    """Residual x + gamma*bv, computed off the critical path;
                    added into the out-projection PSUM via an identity
                    matmul so the final evacuation is a plain ACT copy."""
                    xrb = fpool.tile([P, NCH, state["fb"]], F32R, tag="xrb",
                                     name="xrb")
                    for cc in range(NCH):
                        nc.vector.tensor_scalar(
                            xrb[:, cc, :],
                            x_sb[:, cc, state["isl"]].bitcast(F32),
                            bv_sb[cc][:, 0:1], None, op0=OP.add)
                    state["xrb"] = xrb

                def tail_bc(state):
                    """Broadcast 1/sums to 128 partitions (Pool; SBUF output
                    so the fused zs-scale keeps a single PSUM operand)."""
                    bc_sb = fpool.tile([P, state["fb"]], F32, tag="bc_sb",
                                       name="bc_sb")
                    nc.gpsimd.partition_broadcast(bc_sb[:],
                                                  state["rg"][0:1, :])
                    state["bc"] = bc_sb

                def tail_zs(state):
                    """Fused evacuate+normalize: zs = z * (1/sums)."""
                    state["zs"] = []
                    for cc in range(NCH):
                        t = fpool.tile([P, state["fb"]], F32R, tag=f"zs{cc}",
                                       name=f"zs{cc}")
                        nc.vector.tensor_tensor(t[:], state["z"][cc][:],
                                                state["bc"][:], op=OP.mult)
                        state["zs"].append(t)

                def tail_b(state, last=False):
                    fb = state["fb"]
                    i0 = state["i0"]
                    for co in range(NCH):
                        if co == 1:
                            if last:
                                ops = pse.tile([P, 2 * fb], F32, tag="pe0",
                                               name="opsl")[:, 0:fb]
                            else:
                                ops = psacc.tile([P, fb], F32, tag="smops",
                                                 name="ops2")
                        else:
                            ops = psacc.tile([P, fb], F32, tag="ops",
                                             name="ops")
                        if last:
                            # residual rides the PE; evacuation on the ACT
                            # engine (same table as Exp) keeps the final
                            # serial chain off the DVE
                            nc.tensor.matmul(ops[:], ident_sb,
                                             state["xrb"][:, co, :],
                                             start=True, stop=False)
                        for ci in range(NCH):
                            nc.tensor.matmul(
                                ops[:],
                                wv_sb[:, ci, co * P:(co + 1) * P],
                                state["zs"][ci][:],
                                start=(ci == 0 and not last),
                                stop=(ci == NCH - 1),
                            )
                        o_sb = fpool.tile([P, fb], F32, tag=f"osb{co}",
                                          name="osb")
                        if last:
                            nc.scalar.activation(o_sb[:], ops[:], AF.Copy)
                        else:
                            nc.vector.scalar_tensor_tensor(
                                o_sb[:], ops[:], bv_sb[co][:, 0:1],
                                x_sb[:, co, state["isl"]].bitcast(F32),
                                op0=OP.add, op1=OP.add,
                            )
                        nc.sync.dma_start(
                            d["out"][co * P:(co + 1) * P, i0:i0 + fb],
                            o_sb[:])

                with tc.tile_pool(name="ps_acc", bufs=1,
                                  space="PSUM") as psacc:
                    for isb in range(len(SBS)):
                        if isb == 0:
                            state = states[0]
                        else:
                            state = new_state(isb)
                            states.append(state)
                        state["z"] = [
                            psacc.tile([P, state["fb"]], F32, tag=f"z{cc}",
                                       name=f"z{cc}")
                            for cc in range(NCH)]
                        for g in range(NG):
                            if isb == 0:
                                # groups 0-3 were emitted with the
                                # projections; bodies 0-3 consume their z
                                # and run the remaining k-projections
                                if g < 4:
                                    emit_zg(state, g)
                                    proj_k(4 + g, psacc, "ops")
                                else:
                                    emit_eexp(state, g)
                                    flush_pend(state, g)
                                    if g >= 5:
                                        emit_zg(state, g - 1)
                                continue
                            emit_eexp(state, g)
                            if state["last"] and g == NG - 1:
                                # final superblock: get the last quad's
                                # ones-matmul onto the PE mid-zg so the
                                # recip/bc chain overlaps the trailing z
                                flush_pend(state, g)
                                emit_zg(state, g - 1, jjs=[0, 1])
                                flush_pend(state)
                                emit_zg(state, g - 1, jjs=[2, 3])
                                continue
                            flush_pend(state, g)
                            prev = states[isb - 1]
                            if g == 0:
                                flush_pend(prev)
                                tail_recip(prev)
                                tail_bc(prev)
                                emit_zg(prev, NG - 1)
                                tail_zs(prev)
                            if g >= 1:
                                emit_zg(state, g - 1)
                            if g == 1:
                                tail_b(prev)
                            if g == 2 and state["last"]:
                                emit_xrb(state)
                    last = states[-1]
                    flush_pend(last)
                    tail_recip(last)
                    tail_bc(last)
                    emit_zg(last, NG - 1)
                    tail_zs(last)
                    tail_b(last, last=True)


_programs = {}


def build_program(repeat=1):
    if repeat in _programs:
        return _programs[repeat]
    nc = bacc.Bacc("TRN2", target_bir_lowering=False, debug=False,
                   num_devices=NCORES)
    d = {
        "x": nc.dram_tensor("x", [P, NCH, N], F32R,
                            kind="ExternalInput").ap(),
        "xT": nc.dram_tensor("xT", [N, C], BF16, kind="ExternalInput").ap(),
        "cst": nc.dram_tensor("cst", [P, CPACK], F32R,
                              kind="ExternalInput").ap(),
        "wvT": nc.dram_tensor("wvT", [P, NCH, C], F32R,
                              kind="ExternalInput").ap(),
        "out": nc.dram_tensor("out", [C, NQ], F32, kind="ExternalOutput").ap(),
    }
    with tile.TileContext(nc) as tc:
        for _ in range(repeat):
            _emit_body(nc, tc, d)
    nc.compile()
    _programs[repeat] = nc
    return nc


def make_in_maps(x, Wq, bq, Wk, bk, Wv, bv, gamma):
    x = np.asarray(x, dtype=np.float32)
    Wq = np.asarray(Wq, dtype=np.float32)
    bq = np.asarray(bq, dtype=np.float32)
    Wk = np.asarray(Wk, dtype=np.float32)
    bk = np.asarray(bk, dtype=np.float32)
    Wv = np.asarray(Wv, dtype=np.float32)
    bv = np.asarray(bv, dtype=np.float32)
    gamma = np.asarray(gamma, dtype=np.float32)

    # const pack: wqk cc0 | wqk cc1 | bqk | pad | bvg0 | bvg1 | identity
    cst = np.zeros((P, CPACK), np.float32)
    wqk = np.concatenate([Wq.T, Wk.T], axis=1)          # [256, 64]
    cst[:, 0:64] = wqk[0:P]
    cst[:, 64:128] = wqk[P:C]
    cst[0:2 * CQ, 128] = np.concatenate([bq, bk])
    bvg = gamma.reshape(()) * bv
    cst[:, 130] = bvg[0:P]
    cst[:, 131] = bvg[P:C]
    cst[:, 132:260] = np.eye(P, dtype=np.float32)

    # gamma folded into the value projection weights
    wvt = np.ascontiguousarray(
        (gamma.reshape(()) * Wv).T
        .reshape(NCH, P, C).transpose(1, 0, 2))          # [128, 2, 256]

    shared = {"cst": cst, "wvT": wvt}
    in_maps = []
    for core in range(NCORES):
        b, h = core // 2, core % 2
        xb = x[b].reshape(C, N)
        xr = np.concatenate(
            [xb[:, h * NQ:(h + 1) * NQ], xb[:, (1 - h) * NQ:(2 - h) * NQ]],
            axis=1)
        m = dict(shared)
        m["x"] = np.ascontiguousarray(
            xr.reshape(NCH, P, N).transpose(1, 0, 2))    # [128, 2, 4096]
        m["xT"] = np.ascontiguousarray(xr.T).astype(ml_dtypes.bfloat16)
        in_maps.append(m)
    return in_maps


def assemble_output(results, dtype=np.float32):
    out = np.empty((B, C, N), np.float32)
    for core in range(NCORES):
        b, h = core // 2, core % 2
        out[b][:, h * NQ:(h + 1) * NQ] = results[core]["out"]
    return out.reshape(B, C, HH, WW).astype(dtype, copy=False)


def kernel(x, Wq, bq, Wk, bk, Wv, bv, gamma):
    nc = build_program(repeat=1)
    in_maps = make_in_maps(x, Wq, bq, Wk, bk, Wv, bv, gamma)
    res = run_bass_kernel_spmd(nc, in_maps, list(range(NCORES)))
    return assemble_output(res.results, dtype=np.asarray(x).dtype)
